# revision 20
# baseline (speedup 1.0000x reference)
"""Trainium2 Bass kernel for nn_Block_17738214932786 (spiking transformer block).

Computation (B=16, C=512, N=1024, H=8 heads, HID=2048):
    q = spike(bn(q_w @ x)); k,v likewise          (spikes are 0/1)
    attn = (Qh Kh^T) Vh * 0.25 == Qh (Kh^T Vh) * 0.25   (exact: integers)
    a = spike(attn)                               (threshold attn >= 8)
    a = spike(bn(proj_w @ a + proj_bias))
    x = x + a
    h = spike(bn(fc1_w @ x + fc1_bias))
    h = spike(bn(fc2_w @ h + fc2_bias))
    out = x + h

Strategy: data-parallel over batch across 8 NeuronCores (2 batches/core,
per-core activation matrix [512, 2048]). BatchNorm (training mode: stats
over batch*length) is handled sync-BN style: per-channel [mean, E[y^2]]
AllGathered per conv (tiny buffers); BN+LIF then collapses to a
per-channel threshold compare y >= t.

Precision: qkv convs run as one f32r pass (hi) plus two fp8 DoubleRow
passes (w_lo*x_hi and w_hi*x_lo, operands pre-scaled into fp8 range,
accumulated in a second PSUM bank and combined as y = hi + 2^-16*lo).
This matches 3-pass f32r bit-for-bit on the spike outputs (verified in
emulation + HW probe) at 2/3 the PE cost. proj/fc1/fc2: single-pass
f32r. Attention exact (spikes 0/1; KtV integer counts split hi/lo bf16
losslessly). h1 never leaves SBUF: fc1 slices (4 m-tiles each) are
spiked in place to f32r and immediately consumed by interleaved fc2
partial sweeps accumulating into an SBUF y2 buffer. Emulated end-to-end
rel err 1.23e-2 (gate 2e-2).
"""

import sys
import types
import numpy as np

B, C, N, H = 16, 512, 1024, 8
D = C // H
HID = 4 * C
NCORES = 8
BPC = B // NCORES          # batches per core
COLS = BPC * N             # 2048
P = 128
NKC = C // P               # 4  tiles over C
NMH = HID // P             # 16 tiles over HID
NCH = COLS // 512          # 4  512-col chunks per core
NPT = COLS // P            # 16 col tiles per core
BN_EPS = 1e-5
LO_SCALE = float(2.0 ** -16)

_cache = {}


def _ensure_axon_hooks_shim():
    try:
        import antenv.axon_hooks  # noqa: F401
        return
    except Exception:
        pass
    m = types.ModuleType("antenv.axon_hooks")
    m.get_axon_ntff_profile_hook = lambda: None
    try:
        import antenv  # noqa: F401
    except Exception:
        sys.modules["antenv"] = types.ModuleType("antenv")
    sys.modules["antenv.axon_hooks"] = m


def _build_program():
    from contextlib import ExitStack
    import concourse.bacc as bacc
    import concourse.tile as tile
    from concourse import mybir
    from concourse.masks import make_identity

    dt = mybir.dt
    f32, bf16, f32r = dt.float32, dt.bfloat16, dt.float32r
    f8 = dt.float8e4
    AF = mybir.ActivationFunctionType
    GE = mybir.AluOpType.is_ge
    MUL = mybir.AluOpType.mult
    ADD = mybir.AluOpType.add
    DR = mybir.MatmulPerfMode.DoubleRow
    RG = [list(range(NCORES))]

    nc = bacc.Bacc("TRN2", target_bir_lowering=False, debug=False,
                   num_devices=NCORES)

    xr_in = nc.dram_tensor("x_r", [C, COLS], f32r, kind="ExternalInput")
    x8h_in = nc.dram_tensor("x8h", [C, COLS], f8, kind="ExternalInput")
    x8l_in = nc.dram_tensor("x8l", [C, COLS], f8, kind="ExternalInput")
    wqh_in = nc.dram_tensor("wqkvT_hi", [C, 3 * C], f32r, kind="ExternalInput")
    wq8l_in = nc.dram_tensor("wqkv8l", [C, 3 * C], f8, kind="ExternalInput")
    wq8h_in = nc.dram_tensor("wqkv8h", [C, 3 * C], f8, kind="ExternalInput")
    wp_in = nc.dram_tensor("wprojT", [C, C], f32r, kind="ExternalInput")
    wfc1_in = nc.dram_tensor("wfc1T", [C, HID], f32r, kind="ExternalInput")
    wfc2_in = nc.dram_tensor("wfc2T", [HID, C], f32r, kind="ExternalInput")
    thr_qkv_in = nc.dram_tensor("thr_qkv", [C, 6], f32, kind="ExternalInput")
    thr_proj_in = nc.dram_tensor("thr_proj", [C, 2], f32, kind="ExternalInput")
    thr_fc1_in = nc.dram_tensor("thr_fc1", [HID, 2], f32, kind="ExternalInput")
    thr_fc2_in = nc.dram_tensor("thr_fc2", [C, 2], f32, kind="ExternalInput")
    out_ext = nc.dram_tensor("out", [C, COLS], f32, kind="ExternalOutput")

    def part3(ap, p=P):  # [(m p), n] dram view -> [p, m, n]
        return ap.rearrange("(m p) n -> p m n", p=p)

    with tile.TileContext(nc, pool_alloc_mode="queue") as tc, ExitStack() as es:
        misc = es.enter_context(tc.tile_pool(name="misc", bufs=1))
        dram = es.enter_context(tc.tile_pool(name="dram", bufs=1, space="DRAM"))
        pp_mm = es.enter_context(tc.tile_pool(name="pp_mm", bufs=6, space="PSUM"))
        pp_sm = es.enter_context(tc.tile_pool(name="pp_sm", bufs=2, space="PSUM"))

        ident_bf = misc.tile([P, P], bf16)
        make_identity(nc, ident_bf)
        eps_t = misc.tile([P, 1], f32)
        nc.vector.memset(eps_t, BN_EPS)

        # warmup collective FIRST on the gpsimd queue: the implicit
        # cross-core barrier (~50us) rides on the first collective; with no
        # input deps it issues at t~0 so real AllGathers run warm
        dmy_in = dram.tile([P, 2], f32, name="dmy_in")
        dmy_out = dram.tile([NCORES, P, 2], f32, name="dmy_out")
        nc.gpsimd.collective_compute(
            "AllGather", mybir.AluOpType.bypass, replica_groups=RG,
            ins=[dmy_in.opt()], outs=[dmy_out.opt()])

        par_qkv = misc.tile([P, NKC, 6], f32)
        par_proj = misc.tile([P, NKC, 2], f32)
        par_fc1 = misc.tile([P, NMH, 2], f32)
        par_fc2 = misc.tile([P, NKC, 2], f32)
        neg75 = misc.tile([P, 1], f32)
        nc.vector.memset(neg75, -7.5)

        def stats_finish(name, pool, stats, nm):
            mv = pool.tile([P, nm, 2], f32, name=f"mv_{name}")
            for m in range(nm):
                nc.vector.bn_aggr(out=mv[:, m, :], in_=stats[:, m, :, :])
            pack = pool.tile([P, nm, 2], f32, name=f"pk_{name}")
            nc.vector.tensor_mul(pack[:, :, 1], mv[:, :, 0], mv[:, :, 0])
            nc.vector.tensor_add(pack[:, :, 1], pack[:, :, 1], mv[:, :, 1])
            nc.vector.tensor_copy(pack[:, :, 0], mv[:, :, 0])
            bin_ = dram.tile([P, nm * 2], f32, name=f"arin_{name}")
            bout = dram.tile([NCORES, P, nm * 2], f32, name=f"arout_{name}")
            nc.sync.dma_start(out=bin_, in_=pack)
            nc.gpsimd.collective_compute(
                "AllGather", mybir.AluOpType.bypass, replica_groups=RG,
                ins=[bin_.opt()], outs=[bout.opt()])
            return bout

        def thresholds(name, pool, bout, thr_par, thr_col, nm):
            """AllGathered per-core stats -> reduce -> thresholds [P, nm]."""
            ag = pool.tile([P, NCORES, nm, 2], f32, name=f"ag_{name}")
            nc.sync.dma_start(out=ag, in_=bout.rearrange("r p c -> p r c"))
            nc.vector.tensor_add(ag[:, 0:4], ag[:, 0:4], ag[:, 4:8])
            nc.vector.tensor_add(ag[:, 0:2], ag[:, 0:2], ag[:, 2:4])
            arst = pool.tile([P, nm, 2], f32, name=f"ar_{name}")
            nc.vector.tensor_add(arst, ag[:, 0, :, :], ag[:, 1, :, :])
            nc.vector.tensor_scalar_mul(arst, arst, 1.0 / NCORES)
            t_t = pool.tile([P, nm], f32, name=f"thr_{name}")
            tmp = pool.tile([P, nm], f32, name=f"tmp_{name}")
            nc.vector.tensor_mul(tmp, arst[:, :, 0], arst[:, :, 0])
            nc.vector.tensor_sub(tmp, arst[:, :, 1], tmp)
            nc.scalar.activation(out=tmp, in_=tmp, func=AF.Sqrt,
                                 bias=eps_t, scale=1.0)
            nc.vector.tensor_mul(tmp, tmp, thr_par[:, :, thr_col])
            nc.vector.tensor_add(t_t, tmp, arst[:, :, 0])
            nc.vector.tensor_sub(t_t, t_t, thr_par[:, :, thr_col + 1])
            return t_t

        def spike(dst, src, t_t, tcol, eng=None):
            (eng or nc.vector).tensor_scalar(
                out=dst, in0=src, scalar1=t_t[:, tcol:tcol + 1],
                scalar2=None, op0=GE)

        def transposes(spkb, dstT):
            # bf16 PE transposes, 4 per PSUM bank -> one [P,512] fp8 copy
            for p_ in range(NPT):
                pst = pp_sm.tile([P, 512], bf16, name="ps_sm")
                for m in range(NKC):
                    nc.tensor.transpose(pst[:, P * m:P * m + P],
                                        in_=spkb[:, m, P * p_:P * p_ + P],
                                        identity=ident_bf)
                nc.any.tensor_copy(dstT[:, p_, :], pst)


        def phase_a(a_spk, q_spk, kT, vT):
            with tc.tile_pool(name="p_xin", bufs=1) as p_xin, \
                 tc.tile_pool(name="p_ykv", bufs=2) as p_ykv:
                xr = p_xin.tile([P, NKC, COLS], f32r, name="xr_full")
                x8 = p_xin.tile([P, 2, NKC, COLS], f8)
                # chunk-0 pieces only; the bulk goes behind conv-k's weights
                nc.sync.dma_start(out=xr[:, :, 0:512],
                                  in_=part3(xr_in[:, :])[:, :, 0:512])
                nc.sync.dma_start(out=x8[:, 0, :, 0:512],
                                  in_=part3(x8h_in[:, :])[:, :, 0:512])
                nc.sync.dma_start(out=x8[:, 1, :, 0:512],
                                  in_=part3(x8l_in[:, :])[:, :, 0:512])

                def rest_of_x():
                    for hf in range(1, NCH):
                        cs = slice(512 * hf, 512 * hf + 512)
                        (nc.sync, nc.scalar)[hf % 2].dma_start(
                            out=xr[:, :, cs], in_=part3(xr_in[:, :])[:, :, cs])
                        (nc.scalar, nc.sync)[hf % 2].dma_start(
                            out=x8[:, 0, :, cs],
                            in_=part3(x8h_in[:, :])[:, :, cs])
                        (nc.sync, nc.scalar)[hf % 2].dma_start(
                            out=x8[:, 1, :, cs],
                            in_=part3(x8l_in[:, :])[:, :, cs])
                    nc.scalar.dma_start(out=par_qkv,
                                        in_=part3(thr_qkv_in[:, :]))
                    nc.scalar.dma_start(out=par_proj,
                                        in_=part3(thr_proj_in[:, :]))
                    nc.scalar.dma_start(out=par_fc1,
                                        in_=part3(thr_fc1_in[:, :]))
                    nc.scalar.dma_start(out=par_fc2,
                                        in_=part3(thr_fc2_in[:, :]))
                with tc.tile_pool(name="p_w", bufs=2) as p_w, \
                     tc.tile_pool(name="p_w8", bufs=1) as p_w8:
                    first_conv = [True]

                    def qkv_conv(ci, y_sb):
                        c0 = 512 * ci
                        wh = p_w.tile([P, NKC, C], f32r, name="w_hi", bufs=2)
                        for hf2 in range(2):
                            cc = c0 + 256 * hf2
                            (nc.sync, nc.scalar)[hf2].dma_start(
                                out=wh[:, :, 256 * hf2:256 * hf2 + 256],
                                in_=part3(wqh_in[:, :])[:, :, cc:cc + 256])
                        w8l = p_w8.tile([P, NKC, C], f8, name="w_8l",
                                        bufs=1)
                        nc.scalar.dma_start(
                            out=w8l, in_=part3(wq8l_in[:, :])[:, :, c0:c0 + 512])
                        w8h = p_w8.tile([P, NKC, C], f8, name="w_8h",
                                        bufs=1)
                        nc.sync.dma_start(
                            out=w8h, in_=part3(wq8h_in[:, :])[:, :, c0:c0 + 512])
                        if first_conv[0]:
                            first_conv[0] = False
                            rest_of_x()
                        st = misc.tile([P, NKC, NCH, 6], f32, name=f"st_qkv{ci}")
                        for hf in range(NCH):
                            cs = slice(512 * hf, 512 * hf + 512)
                            for m in range(NKC):
                                ms = slice(P * m, P * m + P)
                                ph = pp_mm.tile([P, 512], f32, name="ps_mm")
                                for k in range(NKC):
                                    nc.tensor.matmul(
                                        ph, lhsT=wh[:, k, ms], rhs=xr[:, k, cs],
                                        start=(k == 0), stop=(k == NKC - 1))
                                pl = pp_mm.tile([P, 512], f32, name="ps_mm")
                                for j in range(2):
                                    nc.tensor.matmul(
                                        pl, lhsT=w8l[:, 2 * j:2 * j + 2, ms],
                                        rhs=x8[:, 0, 2 * j:2 * j + 2, cs],
                                        start=(j == 0), stop=False,
                                        perf_mode=DR)
                                for j in range(2):
                                    nc.tensor.matmul(
                                        pl, lhsT=w8h[:, 2 * j:2 * j + 2, ms],
                                        rhs=x8[:, 1, 2 * j:2 * j + 2, cs],
                                        start=False, stop=(j == 1),
                                        perf_mode=DR)
                                # y = hi + 2^-16*lo; only one PSUM operand
                                # allowed per ALU op: ACT scales lo into
                                # SBUF, then add the hi bank in place
                                nc.scalar.activation(
                                    out=y_sb[:, m, cs], in_=pl,
                                    func=AF.Copy, scale=LO_SCALE)
                                nc.vector.tensor_add(y_sb[:, m, cs],
                                                     y_sb[:, m, cs], ph)
                                nc.vector.bn_stats(out=st[:, m, hf, :],
                                                   in_=y_sb[:, m, cs])
                        return stats_finish(f"qkv{ci}", misc, st, NKC)

                    y_k = p_ykv.tile([P, NKC, COLS], f32, name="ybuf")
                    bout_k = qkv_conv(1, y_k)
                    y_v = p_ykv.tile([P, NKC, COLS], f32, name="ybuf")
                    bout_v = qkv_conv(2, y_v)
                    # k spikes during conv v (AG-k hidden); staged in
                    # q_spk (serially reused: k, then v, then q's own fill)
                    t_k = thresholds("k", misc, bout_k, par_qkv, 2, NKC)
                    for m in range(NKC):
                        spike(q_spk[:, m, :], y_k[:, m, :], t_k, m)
                    y_q = p_ykv.tile([P, NKC, COLS], f32, name="ybuf")
                    bout_q = qkv_conv(0, y_q)
                    transposes(q_spk, kT)
                    # v spikes stage into q_spk (its real fill is last)
                    t_v = thresholds("v", misc, bout_v, par_qkv, 4, NKC)
                    for m in range(NKC):
                        spike(q_spk[:, m, :], y_v[:, m, :], t_v, m)
                    transposes(q_spk, vT)
                    # q spikes last; AG-q end-exposed but QKtV needs q only
                    # after KtV + blockdiag prep
                    t_q = thresholds("q", misc, bout_q, par_qkv, 0, NKC)
                    for m in range(NKC):
                        spike(q_spk[:, m, :], y_q[:, m, :], t_q, m)

        def phase_b(a_spk, q_spk, kT, vT):
            # ---- Phase B: attention (exact integer bf16/fp8) ----
            with tc.tile_pool(name="p_kv", bufs=4) as p_kv:
                kvs = {}
                for b in range(BPC):
                    for j in range(H // 2):   # head pairs -> blockdiag lhsT
                        blk_hi = p_kv.tile([P, P], bf16, name="kvblk_hi")
                        blk_lo = p_kv.tile([P, P], bf16, name="kvblk_lo")
                        nc.gpsimd.memset(blk_hi, 0.0)
                        nc.gpsimd.memset(blk_lo, 0.0)
                        pkv = pp_sm.tile([P, 64], f32, name="ps_sm")
                        for hh in range(2):
                            h_ = 2 * j + hh
                            sl = slice(64 * hh, 64 * hh + 64)
                            for t_ in range(N // P):
                                nc.tensor.matmul(
                                    pkv[sl, :],
                                    lhsT=kT[:, (N // P) * b + t_, D * h_:D * h_ + D],
                                    rhs=vT[:, (N // P) * b + t_, D * h_:D * h_ + D],
                                    start=(t_ == 0), stop=(t_ == N // P - 1),
                                    tile_position=(0, 64 * hh))
                            # lossless integer split: hi=bf16(kv), lo=kv-hi
                            nc.any.tensor_copy(blk_hi[sl, sl], pkv[sl, :])
                            nc.vector.tensor_sub(blk_lo[sl, sl], pkv[sl, :],
                                                 blk_hi[sl, sl])
                        kvs[(b, j)] = (blk_hi, blk_lo)

                for b in range(BPC):
                    for j in range(H // 2):
                        blk_hi, blk_lo = kvs[(b, j)]
                        pas = [pp_mm.tile([P, 512], f32, name="ps_mm")
                               for _ in range(N // 512)]
                        for wi, blk in enumerate((blk_hi, blk_lo)):
                            for n_ in range(N // 512):
                                cs = slice(N * b + 512 * n_, N * b + 512 * n_ + 512)
                                nc.tensor.matmul(pas[n_], lhsT=blk,
                                                 rhs=q_spk[:, j, cs],
                                                 start=(wi == 0), stop=(wi == 1))
                        for n_ in range(N // 512):
                            cs = slice(N * b + 512 * n_, N * b + 512 * n_ + 512)
                            # j 0,1: +-1 spikes via ACT Sign (those wprojT
                            # k-tiles are host-halved; BN threshold algebra
                            # is shift-invariant -> bit-identical). j 2,3:
                            # 0/1 via DVE. Splits the work across engines.
                            if j < 2:
                                nc.scalar.activation(
                                    out=a_spk[:, j, cs], in_=pas[n_],
                                    func=AF.Sign, bias=neg75, scale=1.0)
                            else:
                                nc.vector.tensor_scalar(
                                    out=a_spk[:, j, cs], in0=pas[n_],
                                    scalar1=8.0, scalar2=None, op0=GE)

        def phase_c(a_spk, wpT, xr_res):
            # ---- Phase C: proj (1-pass f32r) + fused spike+residual.
            # Stats AllGather fires PER M-TILE during the conv, so the
            # first thresholds land ~20us earlier and fc1 k-accumulation
            # can begin while later AGs drain.
            # xrr overwrites the a_spk tile (WAR dep after proj matmuls). ----
            with tc.tile_pool(name="p_pr", bufs=1) as p_pr:
                xrr = a_spk

                y_p = p_pr.tile([P, NKC, COLS], f32)
                st_p = misc.tile([P, NKC, NCH, 6], f32, name="st_proj")
                bouts = []
                for m in range(NKC):
                    ms = slice(P * m, P * m + P)
                    pss = [pp_mm.tile([P, 512], f32, name="ps_mm")
                           for _ in range(NCH)]
                    for k in range(NKC):
                        for n_ in range(NCH):
                            nc.tensor.matmul(
                                pss[n_], lhsT=wpT[:, k, ms],
                                rhs=a_spk[:, k, 512 * n_:512 * n_ + 512],
                                start=(k == 0), stop=(k == NKC - 1))
                    for n_ in range(NCH):
                        cs = slice(512 * n_, 512 * n_ + 512)
                        nc.any.tensor_copy(y_p[:, m, cs], pss[n_])
                        nc.vector.bn_stats(out=st_p[:, m, n_, :], in_=pss[n_])
                    bouts.append(stats_finish(
                        f"proj{m}", misc, st_p[:, m:m + 1], 1))
                # PE<->DVE ping-pong keep-alive through the AllGather wait
                wka = p_pr.tile([P, P], bf16, name="wka")
                nc.vector.tensor_copy(wka, ident_bf)
                wpsk = pp_sm.tile([P, P], f32, name="ps_sm")
                for _ in range(14):
                    nc.tensor.matmul(wpsk, lhsT=wka, rhs=wka,
                                     start=True, stop=True)
                    nc.vector.tensor_copy(wka, wpsk)
                for m in range(NKC):
                    t_p = thresholds(f"proj{m}", misc, bouts[m],
                                     par_proj[:, m:m + 1, :], 0, 1)
                    nc.vector.scalar_tensor_tensor(
                        out=xrr[:, m, :], in0=y_p[:, m, :],
                        scalar=t_p[:, 0:1], in1=xr_res[:, m, :],
                        op0=GE, op1=ADD)
            return xrr

        def phase_de(xrr, wfc2T, y2, p_w1, w1s):
            # ====== fc1 slices interleaved with fc2 partial sweeps ======
            # fc1 slice s (4 m-tiles) -> stats AG_s -> spike in place to
            # f32r -> fc2 sweep s accumulates W2[:, slice]*h1[slice] into
            # y2 (SBUF). h1 never exists beyond two live slices.
            NSL = 4
            with tc.tile_pool(name="p_f1a", bufs=1) as p_f1a, \
                 tc.tile_pool(name="p_f1b", bufs=1) as p_f1b, \
                 tc.tile_pool(name="p_tmp", bufs=2) as p_tmp:
                st2 = misc.tile([P, NKC, NCH, 6], f32, name="st_fc2")

                def fc1_slice(s):
                    if s + 2 < NSL:
                        w1n = p_w1.tile([P, NKC, 512], f32r, name="w1q",
                                        bufs=2)
                        (nc.sync, nc.scalar)[s % 2].dma_start(
                            out=w1n,
                            in_=part3(wfc1_in[:, :])[:, :, 512 * (s + 2):512 * (s + 2) + 512])
                        w1s.append(w1n)
                    w1 = w1s[s]
                    y1q = (p_f1a, p_f1b)[s % 2].tile(
                        [P, NKC, COLS], f32r, name="y1q", bufs=1)
                    st_q = misc.tile([P, NKC, NCH, 6], f32, name=f"st_fc1q{s}")
                    for mi in range(NKC):
                        pss = [pp_mm.tile([P, 512], f32, name="ps_mm")
                               for _ in range(NCH)]
                        for k in range(NKC):
                            for n_ in range(NCH):
                                nc.tensor.matmul(
                                    pss[n_],
                                    lhsT=w1[:, k, P * mi:P * mi + P],
                                    rhs=xrr[:, k, 512 * n_:512 * n_ + 512],
                                    start=(k == 0), stop=(k == NKC - 1))
                        for n_ in range(NCH):
                            cs = slice(512 * n_, 512 * n_ + 512)
                            nc.any.tensor_copy(y1q[:, mi, cs], pss[n_])
                            nc.vector.bn_stats(out=st_q[:, mi, n_, :],
                                               in_=pss[n_])
                    return y1q, st_q

                def fc1_finish(s, y1q, bout):
                    t1q = thresholds(f"fc1q{s}", misc, bout,
                                     par_fc1[:, 4 * s:4 * s + 4, :], 0, NKC)
                    nt = misc.tile([P, 2], f32, name=f"nt{s}")
                    nc.vector.tensor_scalar_mul(nt, t1q[:, 0:2], -1.0)
                    for mi in range(NKC):
                        if mi < 2:
                            # +-1 via ACT Sign (those wfc2T k-tiles halved)
                            nc.scalar.activation(
                                out=y1q[:, mi, :], in_=y1q[:, mi, :],
                                func=AF.Sign, bias=nt[:, mi:mi + 1],
                                scale=1.0)
                        else:
                            spike(y1q[:, mi, :], y1q[:, mi, :], t1q, mi)
                    return y1q

                def fc2_sweep(s, h1q):
                    for m in range(NKC):
                        ms = slice(P * m, P * m + P)
                        for n_ in range(NCH):
                            cs = slice(512 * n_, 512 * n_ + 512)
                            ps = pp_mm.tile([P, 512], f32, name="ps_mm")
                            for k in range(NKC):
                                nc.tensor.matmul(
                                    ps, lhsT=wfc2T[:, 4 * s + k, ms],
                                    rhs=h1q[:, k, cs],
                                    start=(k == 0), stop=(k == NKC - 1))
                            if s == 0:
                                nc.any.tensor_copy(y2[:, m, cs], ps)
                            else:
                                if (m + n_) % 2 == 0:
                                    nc.vector.tensor_add(y2[:, m, cs],
                                                         y2[:, m, cs], ps)
                                else:
                                    tmp = p_tmp.tile([P, 512], f32, name="f2t",
                                                     bufs=2)
                                    nc.scalar.activation(out=tmp, in_=ps,
                                                         func=AF.Copy,
                                                         scale=1.0)
                                    nc.gpsimd.tensor_add(y2[:, m, cs],
                                                         y2[:, m, cs], tmp)
                                if s == NSL - 1:
                                    nc.vector.bn_stats(out=st2[:, m, n_, :],
                                                       in_=y2[:, m, cs])

                pend = None
                for s in range(NSL):
                    y1q, st_q = fc1_slice(s)
                    if pend is not None:
                        ps_, py1q, pbout = pend
                        h1q = fc1_finish(ps_, py1q, pbout)
                        fc2_sweep(ps_, h1q)
                    # AG_s emitted AFTER sweep(s-1) so the gpsimd adds are
                    # not queued behind the collective wait
                    bout = stats_finish(f"fc1q{s}", misc, st_q, NKC)
                    pend = (s, y1q, bout)
                ps_, py1q, pbout = pend
                h1q = fc1_finish(ps_, py1q, pbout)
                fc2_sweep(ps_, h1q)

                # ====== tail: fc2 stats -> AG -> fused spike+residual ======
                bout2 = stats_finish("fc2", misc, st2, NKC)
                t2 = thresholds("fc2", misc, bout2, par_fc2, 0, NKC)
                out3 = part3(out_ext[:, :])
                for n_ in range(NCH):
                    cs = slice(512 * n_, 512 * n_ + 512)
                    for m in range(NKC):
                        if (m + n_) % 2 == 0:
                            nc.vector.scalar_tensor_tensor(
                                out=y2[:, m, cs], in0=y2[:, m, cs],
                                scalar=t2[:, m:m + 1], in1=xrr[:, m, cs],
                                op0=GE, op1=ADD)
                        else:
                            nc.vector.tensor_scalar(
                                out=y2[:, m, cs], in0=y2[:, m, cs],
                                scalar1=t2[:, m:m + 1], scalar2=None, op0=GE)
                            nc.gpsimd.tensor_add(y2[:, m, cs], y2[:, m, cs],
                                                 xrr[:, m, cs])
                        (nc.sync, nc.scalar)[(m + n_) % 2].dma_start(
                            out=out3[:, m, cs], in_=y2[:, m, cs])

        with tc.tile_pool(name="p_as", bufs=1) as p_as:  # a_spk/xrr: A..E
            a_spk = p_as.tile([P, NKC, COLS], f32r)
            with tc.tile_pool(name="p_ab", bufs=1) as p_ab:  # lives A..B
                q_spk = p_ab.tile([P, NKC, COLS], bf16)
                kT = p_ab.tile([P, NPT, C], f8)
                vT = p_ab.tile([P, NPT, C], f8)
                phase_a(a_spk, q_spk, kT, vT)
                phase_b(a_spk, q_spk, kT, vT)
            # C/D/E pool: its ring slot lands in the freed phase-A region,
            # so these loads have no WAR dep on attention and stream
            # during it (the DMA queues are idle by then)
            with tc.tile_pool(name="p_cde", bufs=1) as p_cde, \
                 tc.tile_pool(name="p_w1", bufs=2) as p_w1:
                wpT = p_cde.tile([P, NKC, C], f32r)
                nc.sync.dma_start(out=wpT, in_=part3(wp_in[:, :]))
                wfc2T = p_cde.tile([P, NMH, C], f32r)
                for sl_ in range(4):
                    (nc.sync, nc.scalar)[sl_ % 2].dma_start(
                        out=wfc2T[:, 4 * sl_:4 * sl_ + 4, :],
                        in_=part3(wfc2_in[:, :])[:, 4 * sl_:4 * sl_ + 4, :])
                y2 = p_cde.tile([P, NKC, COLS], f32)
                w1s = []
                for s in range(2):
                    w1 = p_w1.tile([P, NKC, 512], f32r, name="w1q", bufs=2)
                    (nc.sync, nc.scalar)[s].dma_start(
                        out=w1,
                        in_=part3(wfc1_in[:, :])[:, :, 512 * s:512 * s + 512])
                    w1s.append(w1)
                with tc.tile_pool(name="p_xr", bufs=1) as p_xr:
                    xr_res = p_xr.tile([P, NKC, COLS], f32r, name="xr_res")
                    for hf in range(NCH):
                        cs = slice(512 * hf, 512 * hf + 512)
                        nc.scalar.dma_start(out=xr_res[:, :, cs],
                                            in_=part3(xr_in[:, :])[:, :, cs])
                    xrr = phase_c(a_spk, wpT, xr_res)
                phase_de(xrr, wfc2T, y2, p_w1, w1s)

    nc.compile()
    return nc


def _f32r(v):
    """Round float32 array to f32r (11-bit mantissa, RNE) - bit-exact vs
    the TRN2 DVE cast (verified on hardware)."""
    x = np.ascontiguousarray(v, np.float32).view(np.uint32)
    keep = np.uint32(0xFFFFF000)
    half = np.uint32(0x800)
    lsb = (x >> np.uint32(12)) & np.uint32(1)
    r = (x + half - np.uint32(1) + lsb) & keep
    return r.view(np.float32)


def build_inputs(inp):
    """Host-side prep: per-core input maps (weights replicated)."""
    import ml_dtypes
    f8 = ml_dtypes.float8_e4m3
    x = inp["x"]

    def thr_pack(g, b, bias):
        A = (2.0 - b) / g
        return np.ascontiguousarray(np.stack([A, bias], axis=1).astype(np.float32))

    wqkvT = np.ascontiguousarray(
        np.concatenate([inp["q_w"].T, inp["k_w"].T, inp["v_w"].T], axis=1))
    wq_hi = _f32r(wqkvT)
    wq_lo = wqkvT - wq_hi
    wq8l = np.ascontiguousarray((wq_lo * 65536.0).astype(f8))
    wq8h = np.ascontiguousarray((wq_hi * 16.0).astype(f8))
    # Per-k-tile scaling: k-tiles whose spikes arrive as +-1 (ACT Sign)
    # get halved weights; 0/1 (DVE) tiles stay full. The BN threshold
    # algebra is shift-invariant, so results are bit-identical.
    wp = _f32r(np.ascontiguousarray(inp["proj_w"].T))
    wp[0:256, :] *= np.float32(0.5)     # j-tiles 0,1 are +-1
    w1 = _f32r(np.ascontiguousarray(inp["fc1_w"].T))
    w2 = _f32r(np.ascontiguousarray(inp["fc2_w"].T))
    w2s = w2.reshape(16, 128, C)
    for kk in range(16):
        if kk % 4 < 2:                   # mi 0,1 of each slice are +-1
            w2s[kk] *= np.float32(0.5)

    zc = np.zeros(C, np.float32)
    thr_qkv = np.ascontiguousarray(np.concatenate([
        thr_pack(inp["q_g"], inp["q_b"], zc),
        thr_pack(inp["k_g"], inp["k_b"], zc),
        thr_pack(inp["v_g"], inp["v_b"], zc)], axis=1))

    shared = dict(
        wqkvT_hi=wq_hi, wqkv8l=wq8l, wqkv8h=wq8h,
        wprojT=wp, wfc1T=w1, wfc2T=w2, thr_qkv=thr_qkv,
        thr_proj=thr_pack(inp["proj_g"], inp["proj_b"], inp["proj_bias"]),
        thr_fc1=thr_pack(inp["fc1_g"], inp["fc1_b"], inp["fc1_bias"]),
        thr_fc2=thr_pack(inp["fc2_g"], inp["fc2_b"], inp["fc2_bias"]))

    in_maps = []
    for i in range(NCORES):
        xl_full = np.ascontiguousarray(
            np.concatenate([x[BPC * i + b] for b in range(BPC)], axis=1))
        x_r = _f32r(xl_full)
        x_lo = xl_full - x_r
        in_maps.append(dict(
            x_r=x_r,
            x8h=np.ascontiguousarray(x_r.astype(f8)),
            x8l=np.ascontiguousarray((x_lo * 4096.0).astype(f8)),
            **shared))
    return in_maps


def get_program():
    if "nc" not in _cache:
        _cache["nc"] = _build_program()
    return _cache["nc"]


def run(in_maps, **kwargs):
    _ensure_axon_hooks_shim()
    from concourse.bass_utils import run_bass_kernel_spmd
    nc = get_program()
    return run_bass_kernel_spmd(nc, in_maps, list(range(NCORES)), **kwargs)


def kernel(**inputs):
    inp = {k: np.asarray(v, dtype=np.float32) for k, v in inputs.items()}
    assert inp["x"].shape == (B, C, N), inp["x"].shape
    res = run(build_inputs(inp))
    out = np.empty((B, C, N), np.float32)
    for i in range(NCORES):
        o = res.results[i]["out"]
        for b in range(BPC):
            out[BPC * i + b] = o[:, N * b:N * (b + 1)]
    return out


# revision 21
# speedup vs baseline: 1.0510x; 1.0510x over previous
"""Trainium2 Bass kernel for nn_Block_17738214932786 (spiking transformer block).

Computation (B=16, C=512, N=1024, H=8 heads, HID=2048):
    q = spike(bn(q_w @ x)); k,v likewise          (spikes are 0/1)
    attn = (Qh Kh^T) Vh * 0.25 == Qh (Kh^T Vh) * 0.25   (exact: integers)
    a = spike(attn)                               (threshold attn >= 8)
    a = spike(bn(proj_w @ a + proj_bias))
    x = x + a
    h = spike(bn(fc1_w @ x + fc1_bias))
    h = spike(bn(fc2_w @ h + fc2_bias))
    out = x + h

Strategy: data-parallel over batch across 8 NeuronCores (2 batches/core,
per-core activation matrix [512, 2048]). BatchNorm (training mode: stats
over batch*length) is handled sync-BN style: per-channel [mean, E[y^2]]
AllGathered per conv (tiny buffers); BN+LIF then collapses to a
per-channel threshold compare y >= t.

Precision: qkv convs run as one f32r pass (hi) plus two fp8 DoubleRow
passes (w_lo*x_hi and w_hi*x_lo, operands pre-scaled into fp8 range,
accumulated in a second PSUM bank and combined as y = hi + 2^-16*lo).
This matches 3-pass f32r bit-for-bit on the spike outputs (verified in
emulation + HW probe) at 2/3 the PE cost. proj/fc1/fc2: single-pass
f32r. Attention exact (spikes 0/1; KtV integer counts split hi/lo bf16
losslessly). h1 never leaves SBUF: fc1 slices (4 m-tiles each) are
spiked in place to f32r and immediately consumed by interleaved fc2
partial sweeps accumulating into an SBUF y2 buffer. Emulated end-to-end
rel err 1.23e-2 (gate 2e-2).
"""

import sys
import types
import numpy as np

B, C, N, H = 16, 512, 1024, 8
D = C // H
HID = 4 * C
NCORES = 8
BPC = B // NCORES          # batches per core
COLS = BPC * N             # 2048
P = 128
NKC = C // P               # 4  tiles over C
NMH = HID // P             # 16 tiles over HID
NCH = COLS // 512          # 4  512-col chunks per core
NPT = COLS // P            # 16 col tiles per core
BN_EPS = 1e-5
LO_SCALE = float(2.0 ** -16)

_cache = {}


def _ensure_axon_hooks_shim():
    try:
        import antenv.axon_hooks  # noqa: F401
        return
    except Exception:
        pass
    m = types.ModuleType("antenv.axon_hooks")
    m.get_axon_ntff_profile_hook = lambda: None
    try:
        import antenv  # noqa: F401
    except Exception:
        sys.modules["antenv"] = types.ModuleType("antenv")
    sys.modules["antenv.axon_hooks"] = m


def _build_program():
    from contextlib import ExitStack
    import concourse.bacc as bacc
    import concourse.tile as tile
    from concourse import mybir
    from concourse.masks import make_identity

    dt = mybir.dt
    f32, bf16, f32r = dt.float32, dt.bfloat16, dt.float32r
    f8 = dt.float8e4
    AF = mybir.ActivationFunctionType
    GE = mybir.AluOpType.is_ge
    MUL = mybir.AluOpType.mult
    ADD = mybir.AluOpType.add
    DR = mybir.MatmulPerfMode.DoubleRow
    RG = [list(range(NCORES))]

    nc = bacc.Bacc("TRN2", target_bir_lowering=False, debug=False,
                   num_devices=NCORES)

    xr_in = nc.dram_tensor("x_r", [C, COLS], f32r, kind="ExternalInput")
    x8h_in = nc.dram_tensor("x8h", [C, COLS], f8, kind="ExternalInput")
    x8l_in = nc.dram_tensor("x8l", [C, COLS], f8, kind="ExternalInput")
    wqh_in = nc.dram_tensor("wqkvT_hi", [C, 3 * C], f32r, kind="ExternalInput")
    wq8l_in = nc.dram_tensor("wqkv8l", [C, 3 * C], f8, kind="ExternalInput")
    wq8h_in = nc.dram_tensor("wqkv8h", [C, 3 * C], f8, kind="ExternalInput")
    wp_in = nc.dram_tensor("wprojT", [C, C], f32r, kind="ExternalInput")
    wfc1_in = nc.dram_tensor("wfc1T", [C, HID], f32r, kind="ExternalInput")
    wfc2_in = nc.dram_tensor("wfc2T", [HID, C], f32r, kind="ExternalInput")
    thr_qkv_in = nc.dram_tensor("thr_qkv", [C, 6], f32, kind="ExternalInput")
    thr_proj_in = nc.dram_tensor("thr_proj", [C, 2], f32, kind="ExternalInput")
    thr_fc1_in = nc.dram_tensor("thr_fc1", [HID, 2], f32, kind="ExternalInput")
    thr_fc2_in = nc.dram_tensor("thr_fc2", [C, 2], f32, kind="ExternalInput")
    out_ext = nc.dram_tensor("out", [C, COLS], f32, kind="ExternalOutput")

    def part3(ap, p=P):  # [(m p), n] dram view -> [p, m, n]
        return ap.rearrange("(m p) n -> p m n", p=p)

    with tile.TileContext(nc, pool_alloc_mode="queue") as tc, ExitStack() as es:
        misc = es.enter_context(tc.tile_pool(name="misc", bufs=1))
        dram = es.enter_context(tc.tile_pool(name="dram", bufs=1, space="DRAM"))
        pp_mm = es.enter_context(tc.tile_pool(name="pp_mm", bufs=6, space="PSUM"))
        pp_sm = es.enter_context(tc.tile_pool(name="pp_sm", bufs=2, space="PSUM"))

        ident_bf = misc.tile([P, P], bf16)
        make_identity(nc, ident_bf)
        eps_t = misc.tile([P, 1], f32)
        nc.vector.memset(eps_t, BN_EPS)

        # warmup collective FIRST on the gpsimd queue: the implicit
        # cross-core barrier (~50us) rides on the first collective; with no
        # input deps it issues at t~0 so real AllGathers run warm
        dmy_in = dram.tile([P, 2], f32, name="dmy_in")
        dmy_out = dram.tile([NCORES, P, 2], f32, name="dmy_out")
        nc.gpsimd.collective_compute(
            "AllGather", mybir.AluOpType.bypass, replica_groups=RG,
            ins=[dmy_in.opt()], outs=[dmy_out.opt()])

        par_qkv = misc.tile([P, NKC, 6], f32)
        par_proj = misc.tile([P, NKC, 2], f32)
        par_fc1 = misc.tile([P, NMH, 2], f32)
        par_fc2 = misc.tile([P, NKC, 2], f32)
        neg75 = misc.tile([P, 1], f32)
        nc.vector.memset(neg75, -7.5)

        def stats_finish(name, pool, stats, nm):
            mv = pool.tile([P, nm, 2], f32, name=f"mv_{name}")
            for m in range(nm):
                nc.vector.bn_aggr(out=mv[:, m, :], in_=stats[:, m, :, :])
            pack = pool.tile([P, nm, 2], f32, name=f"pk_{name}")
            nc.vector.tensor_mul(pack[:, :, 1], mv[:, :, 0], mv[:, :, 0])
            nc.vector.tensor_add(pack[:, :, 1], pack[:, :, 1], mv[:, :, 1])
            nc.vector.tensor_copy(pack[:, :, 0], mv[:, :, 0])
            bin_ = dram.tile([P, nm * 2], f32, name=f"arin_{name}")
            bout = dram.tile([NCORES, P, nm * 2], f32, name=f"arout_{name}")
            nc.sync.dma_start(out=bin_, in_=pack)
            nc.gpsimd.collective_compute(
                "AllGather", mybir.AluOpType.bypass, replica_groups=RG,
                ins=[bin_.opt()], outs=[bout.opt()])
            return bout

        def thresholds(name, pool, bout, thr_par, thr_col, nm):
            """AllGathered per-core stats -> reduce -> thresholds [P, nm]."""
            ag = pool.tile([P, NCORES, nm, 2], f32, name=f"ag_{name}")
            nc.sync.dma_start(out=ag, in_=bout.rearrange("r p c -> p r c"))
            nc.vector.tensor_add(ag[:, 0:4], ag[:, 0:4], ag[:, 4:8])
            nc.vector.tensor_add(ag[:, 0:2], ag[:, 0:2], ag[:, 2:4])
            arst = pool.tile([P, nm, 2], f32, name=f"ar_{name}")
            nc.vector.tensor_add(arst, ag[:, 0, :, :], ag[:, 1, :, :])
            nc.vector.tensor_scalar_mul(arst, arst, 1.0 / NCORES)
            t_t = pool.tile([P, nm], f32, name=f"thr_{name}")
            tmp = pool.tile([P, nm], f32, name=f"tmp_{name}")
            nc.vector.tensor_mul(tmp, arst[:, :, 0], arst[:, :, 0])
            nc.vector.tensor_sub(tmp, arst[:, :, 1], tmp)
            nc.scalar.activation(out=tmp, in_=tmp, func=AF.Sqrt,
                                 bias=eps_t, scale=1.0)
            nc.vector.tensor_mul(tmp, tmp, thr_par[:, :, thr_col])
            nc.vector.tensor_add(t_t, tmp, arst[:, :, 0])
            nc.vector.tensor_sub(t_t, t_t, thr_par[:, :, thr_col + 1])
            return t_t

        def spike(dst, src, t_t, tcol, eng=None):
            (eng or nc.vector).tensor_scalar(
                out=dst, in0=src, scalar1=t_t[:, tcol:tcol + 1],
                scalar2=None, op0=GE)

        def transposes(spkb, dstT):
            # bf16 PE transposes, 4 per PSUM bank -> one [P,512] fp8 copy
            for p_ in range(NPT):
                pst = pp_sm.tile([P, 512], bf16, name="ps_sm")
                for m in range(NKC):
                    nc.tensor.transpose(pst[:, P * m:P * m + P],
                                        in_=spkb[:, m, P * p_:P * p_ + P],
                                        identity=ident_bf)
                nc.any.tensor_copy(dstT[:, p_, :], pst)


        def phase_a(a_spk, q_spk, kT, vT):
            with tc.tile_pool(name="p_xin", bufs=1) as p_xin, \
                 tc.tile_pool(name="p_x", bufs=2) as p_x, \
                 tc.tile_pool(name="p_spk", bufs=1) as p_spk, \
                 tc.tile_pool(name="p_ykv", bufs=2) as p_ykv:
                x8 = p_xin.tile([P, 2, NKC, COLS], f8)
                nc.sync.dma_start(out=x8[:, 0, :, 0:512],
                                  in_=part3(x8h_in[:, :])[:, :, 0:512])
                nc.sync.dma_start(out=x8[:, 1, :, 0:512],
                                  in_=part3(x8l_in[:, :])[:, :, 0:512])

                def rest_of_x():
                    for hf in range(1, NCH):
                        cs = slice(512 * hf, 512 * hf + 512)
                        (nc.scalar, nc.sync)[hf % 2].dma_start(
                            out=x8[:, 0, :, cs],
                            in_=part3(x8h_in[:, :])[:, :, cs])
                        (nc.sync, nc.scalar)[hf % 2].dma_start(
                            out=x8[:, 1, :, cs],
                            in_=part3(x8l_in[:, :])[:, :, cs])
                    nc.scalar.dma_start(out=par_qkv,
                                        in_=part3(thr_qkv_in[:, :]))
                    nc.scalar.dma_start(out=par_proj,
                                        in_=part3(thr_proj_in[:, :]))
                    nc.scalar.dma_start(out=par_fc1,
                                        in_=part3(thr_fc1_in[:, :]))
                    nc.scalar.dma_start(out=par_fc2,
                                        in_=part3(thr_fc2_in[:, :]))
                with tc.tile_pool(name="p_w", bufs=2) as p_w, \
                     tc.tile_pool(name="p_w8", bufs=1) as p_w8:
                    first_conv = [True]

                    def qkv_conv(ci, y_sb):
                        c0 = 512 * ci
                        wh = p_w.tile([P, NKC, C], f32r, name="w_hi", bufs=2)
                        for hf2 in range(2):
                            cc = c0 + 256 * hf2
                            (nc.sync, nc.scalar)[hf2].dma_start(
                                out=wh[:, :, 256 * hf2:256 * hf2 + 256],
                                in_=part3(wqh_in[:, :])[:, :, cc:cc + 256])
                        w8l = p_w8.tile([P, NKC, C], f8, name="w_8l",
                                        bufs=1)
                        nc.scalar.dma_start(
                            out=w8l, in_=part3(wq8l_in[:, :])[:, :, c0:c0 + 512])
                        w8h = p_w8.tile([P, NKC, C], f8, name="w_8h",
                                        bufs=1)
                        nc.sync.dma_start(
                            out=w8h, in_=part3(wq8h_in[:, :])[:, :, c0:c0 + 512])
                        if first_conv[0]:
                            first_conv[0] = False
                            rest_of_x()
                        st = misc.tile([P, NKC, NCH, 6], f32, name=f"st_qkv{ci}")
                        for hf in range(NCH):
                            cs = slice(512 * hf, 512 * hf + 512)
                            xr = p_x.tile([P, NKC, 512], f32r, name="xc",
                                          bufs=2)
                            nc.sync.dma_start(
                                out=xr, in_=part3(xr_in[:, :])[:, :, cs])
                            for m in range(NKC):
                                ms = slice(P * m, P * m + P)
                                ph = pp_mm.tile([P, 512], f32, name="ps_mm")
                                for k in range(NKC):
                                    nc.tensor.matmul(
                                        ph, lhsT=wh[:, k, ms], rhs=xr[:, k, :],
                                        start=(k == 0), stop=(k == NKC - 1))
                                pl = pp_mm.tile([P, 512], f32, name="ps_mm")
                                for j in range(2):
                                    nc.tensor.matmul(
                                        pl, lhsT=w8l[:, 2 * j:2 * j + 2, ms],
                                        rhs=x8[:, 0, 2 * j:2 * j + 2, cs],
                                        start=(j == 0), stop=False,
                                        perf_mode=DR)
                                for j in range(2):
                                    nc.tensor.matmul(
                                        pl, lhsT=w8h[:, 2 * j:2 * j + 2, ms],
                                        rhs=x8[:, 1, 2 * j:2 * j + 2, cs],
                                        start=False, stop=(j == 1),
                                        perf_mode=DR)
                                # y = hi + 2^-16*lo; only one PSUM operand
                                # allowed per ALU op: ACT scales lo into
                                # SBUF, then add the hi bank in place
                                nc.scalar.activation(
                                    out=y_sb[:, m, cs], in_=pl,
                                    func=AF.Copy, scale=LO_SCALE)
                                nc.vector.tensor_add(y_sb[:, m, cs],
                                                     y_sb[:, m, cs], ph)
                                nc.vector.bn_stats(out=st[:, m, hf, :],
                                                   in_=y_sb[:, m, cs])
                        return stats_finish(f"qkv{ci}", misc, st, NKC)

                    y_k = p_ykv.tile([P, NKC, COLS], f32, name="ybuf")
                    bout_k = qkv_conv(1, y_k)
                    y_v = p_ykv.tile([P, NKC, COLS], f32, name="ybuf")
                    bout_v = qkv_conv(2, y_v)
                    # k spikes during conv v (AG-k hidden); staged in spkb
                    t_k = thresholds("k", misc, bout_k, par_qkv, 2, NKC)
                    spkb = p_spk.tile([P, NKC, COLS], bf16, name="spkb",
                                      bufs=1)
                    for m in range(NKC):
                        spike(spkb[:, m, :], y_k[:, m, :], t_k, m)
                    y_q = p_ykv.tile([P, NKC, COLS], f32, name="ybuf")
                    bout_q = qkv_conv(0, y_q)
                    transposes(spkb, kT)
                    # v spikes stage into q_spk (its real fill is last)
                    t_v = thresholds("v", misc, bout_v, par_qkv, 4, NKC)
                    for m in range(NKC):
                        spike(q_spk[:, m, :], y_v[:, m, :], t_v, m)
                    transposes(q_spk, vT)
                    # q spikes last; AG-q end-exposed but QKtV needs q only
                    # after KtV + blockdiag prep
                    t_q = thresholds("q", misc, bout_q, par_qkv, 0, NKC)
                    for m in range(NKC):
                        spike(q_spk[:, m, :], y_q[:, m, :], t_q, m)

        def phase_b(a_spk, q_spk, kT, vT):
            # ---- Phase B: attention (exact integer bf16/fp8) ----
            with tc.tile_pool(name="p_kv", bufs=4) as p_kv:
                kvs = {}
                for b in range(BPC):
                    for j in range(H // 2):   # head pairs -> blockdiag lhsT
                        blk_hi = p_kv.tile([P, P], bf16, name="kvblk_hi")
                        blk_lo = p_kv.tile([P, P], bf16, name="kvblk_lo")
                        nc.gpsimd.memset(blk_hi, 0.0)
                        nc.gpsimd.memset(blk_lo, 0.0)
                        pkv = pp_sm.tile([P, 64], f32, name="ps_sm")
                        for hh in range(2):
                            h_ = 2 * j + hh
                            sl = slice(64 * hh, 64 * hh + 64)
                            for t_ in range(N // P):
                                nc.tensor.matmul(
                                    pkv[sl, :],
                                    lhsT=kT[:, (N // P) * b + t_, D * h_:D * h_ + D],
                                    rhs=vT[:, (N // P) * b + t_, D * h_:D * h_ + D],
                                    start=(t_ == 0), stop=(t_ == N // P - 1),
                                    tile_position=(0, 64 * hh))
                            # lossless integer split: hi=bf16(kv), lo=kv-hi
                            nc.any.tensor_copy(blk_hi[sl, sl], pkv[sl, :])
                            nc.vector.tensor_sub(blk_lo[sl, sl], pkv[sl, :],
                                                 blk_hi[sl, sl])
                        kvs[(b, j)] = (blk_hi, blk_lo)

                for b in range(BPC):
                    for j in range(H // 2):
                        blk_hi, blk_lo = kvs[(b, j)]
                        pas = [pp_mm.tile([P, 512], f32, name="ps_mm")
                               for _ in range(N // 512)]
                        for wi, blk in enumerate((blk_hi, blk_lo)):
                            for n_ in range(N // 512):
                                cs = slice(N * b + 512 * n_, N * b + 512 * n_ + 512)
                                nc.tensor.matmul(pas[n_], lhsT=blk,
                                                 rhs=q_spk[:, j, cs],
                                                 start=(wi == 0), stop=(wi == 1))
                        for n_ in range(N // 512):
                            cs = slice(N * b + 512 * n_, N * b + 512 * n_ + 512)
                            # j 0,1: +-1 spikes via ACT Sign (those wprojT
                            # k-tiles are host-halved; BN threshold algebra
                            # is shift-invariant -> bit-identical). j 2,3:
                            # 0/1 via DVE. Splits the work across engines.
                            if j < 2:
                                nc.scalar.activation(
                                    out=a_spk[:, j, cs], in_=pas[n_],
                                    func=AF.Sign, bias=neg75, scale=1.0)
                            else:
                                nc.vector.tensor_scalar(
                                    out=a_spk[:, j, cs], in0=pas[n_],
                                    scalar1=8.0, scalar2=None, op0=GE)

        def phase_c(a_spk, wpT, xr_res):
            # ---- Phase C: proj (1-pass f32r) + fused spike+residual.
            # xrr overwrites the a_spk tile (WAR dep after proj matmuls). ----
            with tc.tile_pool(name="p_pr", bufs=1) as p_pr:
                xrr = a_spk

                y_p = p_pr.tile([P, NKC, COLS], f32)
                st_p = misc.tile([P, NKC, NCH, 6], f32, name="st_proj")
                for m in range(NKC):
                    ms = slice(P * m, P * m + P)
                    pss = [pp_mm.tile([P, 512], f32, name="ps_mm")
                           for _ in range(NCH)]
                    for k in range(NKC):
                        for n_ in range(NCH):
                            nc.tensor.matmul(
                                pss[n_], lhsT=wpT[:, k, ms],
                                rhs=a_spk[:, k, 512 * n_:512 * n_ + 512],
                                start=(k == 0), stop=(k == NKC - 1))
                    for n_ in range(NCH):
                        cs = slice(512 * n_, 512 * n_ + 512)
                        nc.any.tensor_copy(y_p[:, m, cs], pss[n_])
                        nc.vector.bn_stats(out=st_p[:, m, n_, :], in_=pss[n_])
                bout_p = stats_finish("proj", misc, st_p, NKC)
                # PE<->DVE ping-pong keep-alive through the AllGather wait
                wka = p_pr.tile([P, P], bf16, name="wka")
                nc.vector.tensor_copy(wka, ident_bf)
                wpsk = pp_sm.tile([P, P], f32, name="ps_sm")
                for _ in range(18):
                    nc.tensor.matmul(wpsk, lhsT=wka, rhs=wka,
                                     start=True, stop=True)
                    nc.vector.tensor_copy(wka, wpsk)
                t_p = thresholds("proj", misc, bout_p, par_proj, 0, NKC)
                for m in range(NKC):
                    nc.vector.scalar_tensor_tensor(
                        out=xrr[:, m, :], in0=y_p[:, m, :],
                        scalar=t_p[:, m:m + 1], in1=xr_res[:, m, :],
                        op0=GE, op1=ADD)
            return xrr

        def phase_de(xrr, wfc2T, y2, p_w1, w1s):
            # ====== fc1 slices interleaved with fc2 partial sweeps ======
            # fc1 slice s (4 m-tiles) -> stats AG_s -> spike in place to
            # f32r -> fc2 sweep s accumulates W2[:, slice]*h1[slice] into
            # y2 (SBUF). h1 never exists beyond two live slices.
            NSL = 4
            with tc.tile_pool(name="p_f1a", bufs=1) as p_f1a, \
                 tc.tile_pool(name="p_f1b", bufs=1) as p_f1b, \
                 tc.tile_pool(name="p_tmp", bufs=2) as p_tmp:
                st2 = misc.tile([P, NKC, NCH, 6], f32, name="st_fc2")

                def fc1_slice(s):
                    if s + 2 < NSL:
                        w1n = p_w1.tile([P, NKC, 512], f32r, name="w1q",
                                        bufs=2)
                        (nc.sync, nc.scalar)[s % 2].dma_start(
                            out=w1n,
                            in_=part3(wfc1_in[:, :])[:, :, 512 * (s + 2):512 * (s + 2) + 512])
                        w1s.append(w1n)
                    w1 = w1s[s]
                    y1q = (p_f1a, p_f1b)[s % 2].tile(
                        [P, NKC, COLS], f32r, name="y1q", bufs=1)
                    st_q = misc.tile([P, NKC, NCH, 6], f32, name=f"st_fc1q{s}")
                    for mi in range(NKC):
                        pss = [pp_mm.tile([P, 512], f32, name="ps_mm")
                               for _ in range(NCH)]
                        for k in range(NKC):
                            for n_ in range(NCH):
                                nc.tensor.matmul(
                                    pss[n_],
                                    lhsT=w1[:, k, P * mi:P * mi + P],
                                    rhs=xrr[:, k, 512 * n_:512 * n_ + 512],
                                    start=(k == 0), stop=(k == NKC - 1))
                        for n_ in range(NCH):
                            cs = slice(512 * n_, 512 * n_ + 512)
                            nc.any.tensor_copy(y1q[:, mi, cs], pss[n_])
                            nc.vector.bn_stats(out=st_q[:, mi, n_, :],
                                               in_=pss[n_])
                    return y1q, st_q

                def fc1_finish(s, y1q, bout):
                    t1q = thresholds(f"fc1q{s}", misc, bout,
                                     par_fc1[:, 4 * s:4 * s + 4, :], 0, NKC)
                    nt = misc.tile([P, 2], f32, name=f"nt{s}")
                    nc.vector.tensor_scalar_mul(nt, t1q[:, 0:2], -1.0)
                    for mi in range(NKC):
                        if mi < 2:
                            # +-1 via ACT Sign (those wfc2T k-tiles halved)
                            nc.scalar.activation(
                                out=y1q[:, mi, :], in_=y1q[:, mi, :],
                                func=AF.Sign, bias=nt[:, mi:mi + 1],
                                scale=1.0)
                        else:
                            spike(y1q[:, mi, :], y1q[:, mi, :], t1q, mi)
                    return y1q

                def fc2_sweep(s, h1q):
                    for m in range(NKC):
                        ms = slice(P * m, P * m + P)
                        for n_ in range(NCH):
                            cs = slice(512 * n_, 512 * n_ + 512)
                            ps = pp_mm.tile([P, 512], f32, name="ps_mm")
                            for k in range(NKC):
                                nc.tensor.matmul(
                                    ps, lhsT=wfc2T[:, 4 * s + k, ms],
                                    rhs=h1q[:, k, cs],
                                    start=(k == 0), stop=(k == NKC - 1))
                            if s == 0:
                                nc.any.tensor_copy(y2[:, m, cs], ps)
                            else:
                                if (m + n_) % 2 == 0:
                                    nc.vector.tensor_add(y2[:, m, cs],
                                                         y2[:, m, cs], ps)
                                else:
                                    tmp = p_tmp.tile([P, 512], f32, name="f2t",
                                                     bufs=2)
                                    nc.scalar.activation(out=tmp, in_=ps,
                                                         func=AF.Copy,
                                                         scale=1.0)
                                    nc.gpsimd.tensor_add(y2[:, m, cs],
                                                         y2[:, m, cs], tmp)
                                if s == NSL - 1:
                                    nc.vector.bn_stats(out=st2[:, m, n_, :],
                                                       in_=y2[:, m, cs])

                pend = None
                for s in range(NSL):
                    y1q, st_q = fc1_slice(s)
                    if pend is not None:
                        ps_, py1q, pbout = pend
                        h1q = fc1_finish(ps_, py1q, pbout)
                        fc2_sweep(ps_, h1q)
                    # AG_s emitted AFTER sweep(s-1) so the gpsimd adds are
                    # not queued behind the collective wait
                    bout = stats_finish(f"fc1q{s}", misc, st_q, NKC)
                    pend = (s, y1q, bout)
                ps_, py1q, pbout = pend
                h1q = fc1_finish(ps_, py1q, pbout)
                fc2_sweep(ps_, h1q)

                # ====== tail: fc2 stats -> AG -> fused spike+residual ======
                bout2 = stats_finish("fc2", misc, st2, NKC)
                t2 = thresholds("fc2", misc, bout2, par_fc2, 0, NKC)
                out3 = part3(out_ext[:, :])
                for n_ in range(NCH):
                    cs = slice(512 * n_, 512 * n_ + 512)
                    for m in range(NKC):
                        if (m + n_) % 2 == 0:
                            nc.vector.scalar_tensor_tensor(
                                out=y2[:, m, cs], in0=y2[:, m, cs],
                                scalar=t2[:, m:m + 1], in1=xrr[:, m, cs],
                                op0=GE, op1=ADD)
                        else:
                            nc.vector.tensor_scalar(
                                out=y2[:, m, cs], in0=y2[:, m, cs],
                                scalar1=t2[:, m:m + 1], scalar2=None, op0=GE)
                            nc.gpsimd.tensor_add(y2[:, m, cs], y2[:, m, cs],
                                                 xrr[:, m, cs])
                        (nc.sync, nc.scalar)[(m + n_) % 2].dma_start(
                            out=out3[:, m, cs], in_=y2[:, m, cs])

        with tc.tile_pool(name="p_as", bufs=1) as p_as:  # a_spk/xrr: A..E
            a_spk = p_as.tile([P, NKC, COLS], f32r)
            with tc.tile_pool(name="p_ab", bufs=1) as p_ab:  # lives A..B
                q_spk = p_ab.tile([P, NKC, COLS], bf16)
                kT = p_ab.tile([P, NPT, C], f8)
                vT = p_ab.tile([P, NPT, C], f8)
                phase_a(a_spk, q_spk, kT, vT)
                phase_b(a_spk, q_spk, kT, vT)
            # C/D/E pool: its ring slot lands in the freed phase-A region,
            # so these loads have no WAR dep on attention and stream
            # during it (the DMA queues are idle by then)
            with tc.tile_pool(name="p_cde", bufs=1) as p_cde, \
                 tc.tile_pool(name="p_w1", bufs=2) as p_w1:
                wpT = p_cde.tile([P, NKC, C], f32r)
                nc.sync.dma_start(out=wpT, in_=part3(wp_in[:, :]))
                wfc2T = p_cde.tile([P, NMH, C], f32r)
                for sl_ in range(4):
                    (nc.sync, nc.scalar)[sl_ % 2].dma_start(
                        out=wfc2T[:, 4 * sl_:4 * sl_ + 4, :],
                        in_=part3(wfc2_in[:, :])[:, 4 * sl_:4 * sl_ + 4, :])
                y2 = p_cde.tile([P, NKC, COLS], f32)
                w1s = []
                for s in range(2):
                    w1 = p_w1.tile([P, NKC, 512], f32r, name="w1q", bufs=2)
                    (nc.sync, nc.scalar)[s].dma_start(
                        out=w1,
                        in_=part3(wfc1_in[:, :])[:, :, 512 * s:512 * s + 512])
                    w1s.append(w1)
                with tc.tile_pool(name="p_xr", bufs=1) as p_xr:
                    xr_res = p_xr.tile([P, NKC, COLS], f32r, name="xr_res")
                    for hf in range(NCH):
                        cs = slice(512 * hf, 512 * hf + 512)
                        nc.scalar.dma_start(out=xr_res[:, :, cs],
                                            in_=part3(xr_in[:, :])[:, :, cs])
                    xrr = phase_c(a_spk, wpT, xr_res)
                phase_de(xrr, wfc2T, y2, p_w1, w1s)

    nc.compile()
    return nc


def _f32r(v):
    """Round float32 array to f32r (11-bit mantissa, RNE) - bit-exact vs
    the TRN2 DVE cast (verified on hardware)."""
    x = np.ascontiguousarray(v, np.float32).view(np.uint32)
    keep = np.uint32(0xFFFFF000)
    half = np.uint32(0x800)
    lsb = (x >> np.uint32(12)) & np.uint32(1)
    r = (x + half - np.uint32(1) + lsb) & keep
    return r.view(np.float32)


def build_inputs(inp):
    """Host-side prep: per-core input maps (weights replicated)."""
    import ml_dtypes
    f8 = ml_dtypes.float8_e4m3
    x = inp["x"]

    def thr_pack(g, b, bias):
        A = (2.0 - b) / g
        return np.ascontiguousarray(np.stack([A, bias], axis=1).astype(np.float32))

    wqkvT = np.ascontiguousarray(
        np.concatenate([inp["q_w"].T, inp["k_w"].T, inp["v_w"].T], axis=1))
    wq_hi = _f32r(wqkvT)
    wq_lo = wqkvT - wq_hi
    wq8l = np.ascontiguousarray((wq_lo * 65536.0).astype(f8))
    wq8h = np.ascontiguousarray((wq_hi * 16.0).astype(f8))
    # Per-k-tile scaling: k-tiles whose spikes arrive as +-1 (ACT Sign)
    # get halved weights; 0/1 (DVE) tiles stay full. The BN threshold
    # algebra is shift-invariant, so results are bit-identical.
    wp = _f32r(np.ascontiguousarray(inp["proj_w"].T))
    wp[0:256, :] *= np.float32(0.5)     # j-tiles 0,1 are +-1
    w1 = _f32r(np.ascontiguousarray(inp["fc1_w"].T))
    w2 = _f32r(np.ascontiguousarray(inp["fc2_w"].T))
    w2s = w2.reshape(16, 128, C)
    for kk in range(16):
        if kk % 4 < 2:                   # mi 0,1 of each slice are +-1
            w2s[kk] *= np.float32(0.5)

    zc = np.zeros(C, np.float32)
    thr_qkv = np.ascontiguousarray(np.concatenate([
        thr_pack(inp["q_g"], inp["q_b"], zc),
        thr_pack(inp["k_g"], inp["k_b"], zc),
        thr_pack(inp["v_g"], inp["v_b"], zc)], axis=1))

    shared = dict(
        wqkvT_hi=wq_hi, wqkv8l=wq8l, wqkv8h=wq8h,
        wprojT=wp, wfc1T=w1, wfc2T=w2, thr_qkv=thr_qkv,
        thr_proj=thr_pack(inp["proj_g"], inp["proj_b"], inp["proj_bias"]),
        thr_fc1=thr_pack(inp["fc1_g"], inp["fc1_b"], inp["fc1_bias"]),
        thr_fc2=thr_pack(inp["fc2_g"], inp["fc2_b"], inp["fc2_bias"]))

    in_maps = []
    for i in range(NCORES):
        xl_full = np.ascontiguousarray(
            np.concatenate([x[BPC * i + b] for b in range(BPC)], axis=1))
        x_r = _f32r(xl_full)
        x_lo = xl_full - x_r
        in_maps.append(dict(
            x_r=x_r,
            x8h=np.ascontiguousarray(x_r.astype(f8)),
            x8l=np.ascontiguousarray((x_lo * 4096.0).astype(f8)),
            **shared))
    return in_maps


def get_program():
    if "nc" not in _cache:
        _cache["nc"] = _build_program()
    return _cache["nc"]


def run(in_maps, **kwargs):
    _ensure_axon_hooks_shim()
    from concourse.bass_utils import run_bass_kernel_spmd
    nc = get_program()
    return run_bass_kernel_spmd(nc, in_maps, list(range(NCORES)), **kwargs)


def kernel(**inputs):
    inp = {k: np.asarray(v, dtype=np.float32) for k, v in inputs.items()}
    assert inp["x"].shape == (B, C, N), inp["x"].shape
    res = run(build_inputs(inp))
    out = np.empty((B, C, N), np.float32)
    for i in range(NCORES):
        o = res.results[i]["out"]
        for b in range(BPC):
            out[BPC * i + b] = o[:, N * b:N * (b + 1)]
    return out
